# revision 10
# baseline (speedup 1.0000x reference)
"""Cross-attention (1x1-conv QKV + softmax attention + residual) on 8 TRN2 cores.

Sharding: batch (4) x query-half (2) -> 8 shards, one per core. Each core
computes attention for 2048 query tokens of one batch against all 4096
key tokens of that batch, entirely in channel-major [C, N] layout:

  qT = wq^T @ x1_half + bq            [C, 2048]   (bf16)
  kT = wk^T @ x2                      [C, 4096]   (bf16; bk dropped — a
                                       per-query-constant logit shift
                                       cancels exactly in softmax)
  v  = x2^T @ wv^T (token-major)      [4096, C]   (fp8, bias folded later)
  S^T tiles = kT_chunk^T @ qT         [128j, 512i] (PSUM f32)
  P = exp(S^T * 1/sqrt(C))            (ACT exp LUT / DVE Schraudolph split,
                                       no max-subtraction: |S*scale| ~ 2)
  O  += v_chunk^T @ P_chunk           [C, 512i]   (PSUM f32, DoubleRow fp8)
  den: Dsum[p,q] += P_chunk[p,q]      (DVE + Pool elementwise over pairs,
                                       SBUF bf16 — keeps most ones-matmuls
                                       off the PE) then bf16 ones-matmuls
                                       reduce partitions + broadcast
  out = O * (1/den) + bv + x1_half    (reciprocal+mul on DVE, residual add
                                       on Pool; v-bias exact after softmax
                                       normalization)

Per-block epilogues are emitted lazily (mid next block) so the PE queue
never stalls on the vector-den tail.
"""

import os
import sys

import numpy as np

os.environ.setdefault("JAX_COMPILATION_CACHE_DIR", "/tmp/jaxcache")


def _ensure_concourse():
    try:
        import concourse  # noqa: F401
        return
    except ImportError:
        pass
    for p in ("/opt/trn_rl_repo", os.path.expanduser("~/.axon_site/_ro/trn_rl_repo")):
        if os.path.isdir(p):
            sys.path.insert(0, p)
            try:
                import concourse  # noqa: F401
                return
            except ImportError:
                sys.path.remove(p)
    raise ImportError("concourse (bass) not importable")


_ensure_concourse()

import concourse.bass as bass  # noqa: E402
import concourse.mybir as mybir  # noqa: E402
import concourse.tile as tile  # noqa: E402
from concourse import bacc  # noqa: E402
from concourse.bass_utils import run_bass_kernel_spmd  # noqa: E402

F32 = mybir.dt.float32
F32R = mybir.dt.float32r
BF16 = mybir.dt.bfloat16
FP8 = mybir.dt.float8e4

C = 128          # channels / hidden dim
B = 4            # batch
N = 4096         # tokens per batch (64*64)
NQ = 2048        # query tokens per core (half batch)
N_CORES = 8
NJT = N // 128   # 32 key chunks of 128
N_IB = NQ // 512  # 4 query blocks of 512
NG = NJT // 2    # 16 key-chunk pairs
SCALE = float(1.0 / np.sqrt(C))

# Schraudolph exp on DVE: i8 = S*EXPA + EXPB, bit pattern read as fp8e4m3
# approximates exp(S*SCALE) (3-bit-mantissa log-linear interpolation).
# EXPB includes +0.5 so numpy-trunc in CoreSim == round-to-nearest on HW
# minus half an ulp; softmax normalization cancels the systematic factor.
EXPA = float(8.0 * SCALE / np.log(2.0))
EXPB = 56.5

# den pair-accumulation engine split per block (16 key-chunk pairs):
# Pool (gpsimd, SBUF-only, otherwise idle) and DVE accumulate e pairs
# elementwise into per-engine Dsum tiles; a few pairs stay on the PE as
# fp8 DoubleRow ones-matmuls. Block 0 keeps the PE out of den so the
# PSUM aux rotation stays free for the setup-phase k/v staging.
POOL_DEN = frozenset((0, 2, 4, 6, 9, 11, 13))
PE_DEN = frozenset((5, 15))


def exp_engine(ib, j):
    """Engine for the exp of key half-chunk j (0..31) of query block ib.

    ACT (exact exp LUT) and DVE (Schraudolph tensor_scalar) split the
    softmax exp work; ACT carries the setup-phase k/v/q conversions during
    block 0, so it gets a smaller share there; DVE carries den-adds and the
    epilogues, so ACT takes most exps in blocks 1-3.
    """
    if ib == 0:
        return "dve" if j % 2 == 1 else "act"
    return "dve" if j % 4 == 2 else "act"


def build_nc(repeats=1):
    nc = bacc.Bacc("TRN2", target_bir_lowering=False, debug=False,
                   num_devices=N_CORES)

    x1bf = nc.declare_dram_parameter("x1bf", [C, NQ], BF16, isOutput=False)
    x2f = nc.declare_dram_parameter("x2f", [C, N], BF16, isOutput=False)
    wqT = nc.declare_dram_parameter("wqT", [C, C], BF16, isOutput=False)
    wkT = nc.declare_dram_parameter("wkT", [C, C], BF16, isOutput=False)
    wvT = nc.declare_dram_parameter("wvT", [C, C], BF16, isOutput=False)
    bqv = nc.declare_dram_parameter("bqv", [C, 2], F32, isOutput=False)
    out = nc.declare_dram_parameter("out", [C, NQ], F32, isOutput=True)

    with tile.TileContext(nc) as tc:
        with (
            tc.tile_pool(name="const", bufs=1) as cpool,
            tc.tile_pool(name="persist", bufs=1) as ppool,
            tc.tile_pool(name="work", bufs=2) as wpool,
            tc.tile_pool(name="ps_s", bufs=4, space="PSUM") as ps_s,
            tc.tile_pool(name="ps_o", bufs=2, space="PSUM") as ps_o,
            tc.tile_pool(name="ps_aux", bufs=2, space="PSUM") as ps_aux,
        ):
            pools = (cpool, ppool, wpool, ps_s, ps_o, ps_aux)
            if repeats == 1:
                _build_body(nc, pools, x1bf, x2f, wqT, wkT, wvT, bqv, out)
            else:
                hints = (mybir.EngineType.PE, mybir.EngineType.Activation,
                         mybir.EngineType.DVE, mybir.EngineType.SP,
                         mybir.EngineType.Pool)
                with tc.For_i(0, repeats, 1, hint_engines=hints):
                    _build_body(nc, pools, x1bf, x2f, wqT, wkT, wvT, bqv,
                                out)

    nc.compile()
    return nc


def _build_body(nc, pools, x1bf, x2f, wqT, wkT, wvT, bqv, out):
    (cpool, ppool, wpool, ps_s, ps_o, ps_aux) = pools
    LAG = int(os.environ.get("K_LAG", "3"))
    DIAG_NO_DEN = os.environ.get("K_NO_DEN") == "1"
    DIAG_NO_PV = os.environ.get("K_NO_PV") == "1"
    DIAG_NO_EXP = os.environ.get("K_NO_EXP") == "1"
    DIAG_NO_S = os.environ.get("K_NO_S") == "1"

    # ---- constants: weights + biases + f32r ones for den broadcast -----
    w_b = {}
    for wname, wdram in (("wq", wqT), ("wk", wkT), ("wv", wvT)):
        wb = cpool.tile([C, C], BF16, tag=f"{wname}b", name=f"{wname}b")
        nc.sync.dma_start(wb[:], wdram[:])
        w_b[wname] = wb
    ones_r = cpool.tile([C, C], BF16, tag="ones_r", name="ones_r")
    nc.vector.memset(ones_r[:], 1.0)
    # DoubleRow ones with M=128 for the PE-resident den pairs: the DR
    # matmul then writes the denominator pre-broadcast across partitions.
    ones_dr = cpool.tile([C, 2, C], FP8, tag="ones_dr", name="ones_dr")
    nc.vector.memset(ones_dr[:].rearrange("p two n -> p (two n)"), 1.0)
    bias_t = cpool.tile([C, 2], F32, tag="bias", name="bias_t")
    bias_sb = {"bq": bias_t[:, 0:1], "bv": bias_t[:, 1:2]}

    # ---- input DMAs: x2 chunk ci feeds k/v; x1 chunk ib feeds q(ib) ----
    x1b = [None] * N_IB
    x2b = [None] * 8
    order = [("x2", 0), ("x1", 0), ("x2", 1), ("x2", 2), ("x1", 1),
             ("x2", 3), ("x2", 4), ("x1", 2), ("x2", 5), ("x2", 6),
             ("x1", 3), ("x2", 7)]
    bias_loaded = [False]
    for kind, ci in order:
        if kind == "x1":
            xb = ppool.tile([C, 512], BF16, tag=f"x1b{ci}", name=f"x1b{ci}")
            nc.sync.dma_start(xb[:], x1bf[:, ci * 512:(ci + 1) * 512])
            x1b[ci] = xb
        else:
            xb = ppool.tile([C, 512], BF16, tag=f"x2b{ci}", name=f"x2b{ci}")
            nc.sync.dma_start(xb[:], x2f[:, ci * 512:(ci + 1) * 512])
            x2b[ci] = xb
        if not bias_loaded[0] and ci >= 1:
            nc.sync.dma_start(bias_t[:], bqv[:])
            bias_loaded[0] = True

    qb = [None] * N_IB
    x1c = [None] * N_IB
    kb = []
    vpair = []

    def emit_qproj(ib):
        q_ps = ps_o.tile([C, 512], F32, tag="o", name=f"qps{ib}")
        nc.tensor.matmul(q_ps[:], w_b["wq"][:], x1b[ib][:],
                         start=True, stop=True)
        qt = ppool.tile([C, 512], BF16, tag=f"qb{ib}", name=f"qb{ib}")
        nc.scalar.activation(qt[:], q_ps[:],
                             mybir.ActivationFunctionType.Identity,
                             bias=bias_sb["bq"], scale=1.0)
        qb[ib] = qt

    def emit_x1c(ib):
        # residual base + v-bias in f32 (v-bias exact after normalization)
        xc = ppool.tile([C, 512], F32, tag=f"x1c{ib}", name=f"x1c{ib}")
        nc.scalar.activation(xc[:], x1b[ib][:],
                             mybir.ActivationFunctionType.Identity,
                             bias=bias_sb["bv"], scale=1.0)
        x1c[ib] = xc

    # pending epilogue of the previous block, injected mid-next-block so
    # the PE queue never waits on the vector-den tail
    pending = [None]

    def attention_ib(ib):
        if qb[ib] is None:
            emit_qproj(ib)
        o_ps = ps_o.tile([C, 512], F32, tag="o", name=f"ops{ib}")
        if DIAG_NO_PV:
            nc.vector.memset(o_ps[:], 1.0)
        epairs = {}
        dsum = {}  # engine -> Dsum tile [128, 2, 512] f32
        den_state = {"ps": None, "started": False}

        def den_pair(jg):
            if DIAG_NO_DEN:
                return
            e_flat = epairs[jg][:].rearrange("p two n -> p (two n)")
            if ib > 0 and jg in PE_DEN:
                if den_state["ps"] is None:
                    den_state["ps"] = ps_aux.tile(
                        [C, 512], F32, tag="aux", name=f"den{ib}")
                nc.tensor.matmul(
                    den_state["ps"][:], ones_dr[:], epairs[jg][:],
                    start=not den_state["started"], stop=False,
                    perf_mode=mybir.MatmulPerfMode.DoubleRow)
                den_state["started"] = True
                return
            eng = "pool" if jg in POOL_DEN else "dve"
            if eng not in dsum:
                t = wpool.tile([128, 2, 512], BF16, tag=f"ds_{eng}", bufs=2,
                               name=f"ds_{eng}{ib}")
                dsum[eng] = t
                t_flat = t[:].rearrange("p two n -> p (two n)")
                if eng == "pool":
                    nc.gpsimd.tensor_copy(t_flat, e_flat)
                else:
                    nc.vector.tensor_copy(t_flat, e_flat)
            else:
                t = dsum[eng]
                t_flat = t[:].rearrange("p two n -> p (two n)")
                if eng == "pool":
                    nc.gpsimd.tensor_add(t_flat, t_flat, e_flat)
                else:
                    nc.vector.tensor_add(t_flat, t_flat, e_flat)

        def emit_s(jg):
            e = wpool.tile([128, 2, 512], FP8, tag="e",
                           bufs=20, name=f"e{ib}_{jg}")
            epairs[jg] = e
            if DIAG_NO_S:
                nc.vector.memset(
                    e[:].rearrange("p two n -> p (two n)"), 1.0)
                return
            for half in range(2):
                j = 2 * jg + half
                s_ps = ps_s.tile([128, 512], F32, tag="s",
                                 name=f"sps{ib}_{j}")
                kt = kb[j // 4][:, (j % 4) * 128:(j % 4) * 128 + 128]
                nc.tensor.matmul(s_ps[:], kt, qb[ib][:],
                                 start=True, stop=True)
                if DIAG_NO_EXP:
                    nc.vector.memset(e[:, half, :].bitcast(mybir.dt.int8), 1)
                    continue
                if exp_engine(ib, j) == "dve":
                    nc.vector.tensor_scalar(
                        e[:, half, :].bitcast(mybir.dt.int8),
                        s_ps[:], EXPA, EXPB,
                        mybir.AluOpType.mult, mybir.AluOpType.add)
                else:
                    nc.scalar.activation(
                        e[:, half, :], s_ps[:],
                        mybir.ActivationFunctionType.Exp,
                        bias=0.0, scale=SCALE)

        def emit_pv(jg):
            if DIAG_NO_PV:
                return
            nc.tensor.matmul(
                o_ps[:], vpair[jg], epairs[jg][:],
                start=jg == 0, stop=jg == NG - 1,
                perf_mode=mybir.MatmulPerfMode.DoubleRow)

        for jg in range(NG):
            emit_s(jg)
            den_pair(jg)
            if jg >= LAG:
                emit_pv(jg - LAG)
            if jg == 4:
                if x1c[ib] is None:
                    emit_x1c(ib)
                if pending[0] is not None:
                    pending[0]()
                    pending[0] = None
            yield
        for jg in range(NG - LAG, NG):
            emit_pv(jg)
        for jg in range(NG):
            del epairs[jg]

        def epilogue():
            den_ps = den_state["ps"]
            if den_ps is None:
                den_ps = ps_aux.tile([C, 512], F32, tag="aux",
                                     name=f"den{ib}")
            if DIAG_NO_DEN:
                nc.vector.memset(den_ps[:], 1.0)
            else:
                # partition-reduce + broadcast each engine Dsum with an
                # f32r ones-matmul (1 cyc/col at 512 cols), accumulating
                # on top of any PE-resident DoubleRow den pairs
                parts = []
                for eng in ("dve", "pool"):
                    if eng in dsum:
                        parts.append(dsum[eng])
                last = (len(parts) - 1, 1)
                for pi, t in enumerate(parts):
                    for h in range(2):
                        nc.tensor.matmul(
                            den_ps[:], ones_r[:], t[:, h, :],
                            start=not den_state["started"],
                            stop=(pi, h) == last,
                            skip_group_check=True)
                        den_state["started"] = True
            rbs = wpool.tile([C, 512], F32, tag="rbs", bufs=2,
                             name=f"rbs{ib}")
            nc.vector.reciprocal(rbs[:], den_ps[:])
            ob = wpool.tile([C, 512], F32, tag="ob", bufs=2, name=f"ob{ib}")
            nc.vector.tensor_mul(ob[:], o_ps[:], rbs[:])
            nc.gpsimd.tensor_add(ob[:], ob[:], x1c[ib][:])
            nc.sync.dma_start(out[:, ib * 512:(ib + 1) * 512], ob[:])

        pending[0] = epilogue
        yield

    # ---- setup: per x2 chunk, k projection + v tiles, advancing block 0
    # attention as chunks land (pair jg needs k/v subchunks 2jg, 2jg+1)
    gen0_holder = [None]
    for ci in range(8):
        k_ps = ps_aux.tile([C, 512], F32, tag="aux", name=f"kps{ci}")
        nc.tensor.matmul(k_ps[:], w_b["wk"][:], x2b[ci][:],
                         start=True, stop=True)
        kt = ppool.tile([C, 512], BF16, tag=f"kb{ci}", name=f"kb{ci}")
        nc.scalar.activation(kt[:], k_ps[:],
                             mybir.ActivationFunctionType.Copy,
                             bias=0.0, scale=1.0)
        kb.append(kt)
        # v chunks of this ci as one PSUM quad [128, 4, C] (1 bank),
        # converted to fp8 with a single wide ACT copy
        v_ps = ps_aux.tile([128, 4, C], F32, tag="aux", name=f"vps{ci}")
        for t in range(4):
            lhs = x2b[ci][:, t * 128:t * 128 + 128]
            nc.tensor.matmul(v_ps[:, t, :], lhs, w_b["wv"][:],
                             start=True, stop=True)
        vq = ppool.tile([128, 4, C], FP8, tag=f"vq{ci}", name=f"vq{ci}")
        nc.scalar.activation(vq[:], v_ps[:],
                             mybir.ActivationFunctionType.Copy,
                             bias=0.0, scale=1.0)
        vpair.append(vq[:, 0:2, :])
        vpair.append(vq[:, 2:4, :])
        if gen0_holder[0] is None:
            gen0_holder[0] = attention_ib(0)
        next(gen0_holder[0], None)
        next(gen0_holder[0], None)

    # ---- attention main loop --------------------------------------------
    gen0 = gen0_holder[0]
    if gen0 is not None:
        for _ in gen0:
            pass
    for ib2 in (1, 2, 3):
        for _ in attention_ib(ib2):
            pass
    if pending[0] is not None:
        pending[0]()
        pending[0] = None


_NC_CACHE = None


def _get_nc():
    global _NC_CACHE
    if _NC_CACHE is None:
        _NC_CACHE = build_nc()
    return _NC_CACHE


def make_in_maps(x1, x2, wq, bq, wk, bk, wv, bv):
    x1 = np.asarray(x1, np.float32)
    x2 = np.asarray(x2, np.float32)
    t1 = np.ascontiguousarray(x1.reshape(B, C, N))
    t2 = np.ascontiguousarray(x2.reshape(B, C, N))
    import ml_dtypes
    bf = ml_dtypes.bfloat16
    shared = {
        "wqT": np.ascontiguousarray(np.asarray(wq, np.float32).T.astype(bf)),
        "wkT": np.ascontiguousarray(np.asarray(wk, np.float32).T.astype(bf)),
        "wvT": np.ascontiguousarray(np.asarray(wv, np.float32).T.astype(bf)),
        "bqv": np.ascontiguousarray(np.stack(
            [np.asarray(bq, np.float32), np.asarray(bv, np.float32)],
            axis=1)),
    }
    in_maps = []
    for core in range(N_CORES):
        b, h = core // 2, core % 2
        in_maps.append({
            "x1bf": np.ascontiguousarray(
                t1[b][:, h * NQ:(h + 1) * NQ]).astype(bf),
            "x2f": t2[b].astype(bf),
            **shared,
        })
    return in_maps


def assemble_out(results):
    out = np.empty((B, C, N), np.float32)
    for core in range(N_CORES):
        b, h = core // 2, core % 2
        out[b][:, h * NQ:(h + 1) * NQ] = results[core]["out"]
    return out.reshape(B, C, 64, 64)


def kernel(x1, x2, wq, bq, wk, bk, wv, bv):
    nc = _get_nc()
    in_maps = make_in_maps(x1, x2, wq, bq, wk, bk, wv, bv)
    res = run_bass_kernel_spmd(nc, in_maps, list(range(N_CORES)))
    return assemble_out(res.results)


# revision 14
# speedup vs baseline: 1.0588x; 1.0588x over previous
"""Cross-attention (1x1-conv QKV + softmax attention + residual) on 8 TRN2 cores.

Sharding: batch (4) x query-half (2) -> 8 shards, one per core. Each core
computes attention for 2048 query tokens of one batch against all 4096
key tokens of that batch, entirely in channel-major [C, N] layout:

  qT = wq^T @ x1_half + bq            [C, 2048]   (bf16)
  kT = wk^T @ x2                      [C, 4096]   (bf16; bk dropped — a
                                       per-query-constant logit shift
                                       cancels exactly in softmax)
  v  = x2^T @ wv^T (token-major)      [4096, C]   (fp8, bias folded later)
  S^T tiles = kT_chunk^T @ qT         [128j, 512i] (PSUM f32)
  P = exp(S^T * 1/sqrt(C))            (ACT exp LUT / DVE Schraudolph split,
                                       no max-subtraction: |S*scale| ~ 2)
  O  += v_chunk^T @ P_chunk           [C, 512i]   (PSUM f32, DoubleRow fp8)
  den: Dsum[p,q] += P_chunk[p,q]      (DVE + Pool elementwise over pairs,
                                       SBUF bf16 — keeps most ones-matmuls
                                       off the PE) then bf16 ones-matmuls
                                       reduce partitions + broadcast
  out = O * (1/den) + bv + x1_half    (reciprocal+mul on DVE, residual add
                                       on Pool; v-bias exact after softmax
                                       normalization)

Per-block epilogues are emitted lazily (mid next block) so the PE queue
never stalls on the vector-den tail.
"""

import os
import sys

import numpy as np

os.environ.setdefault("JAX_COMPILATION_CACHE_DIR", "/tmp/jaxcache")


def _ensure_concourse():
    try:
        import concourse  # noqa: F401
        return
    except ImportError:
        pass
    for p in ("/opt/trn_rl_repo", os.path.expanduser("~/.axon_site/_ro/trn_rl_repo")):
        if os.path.isdir(p):
            sys.path.insert(0, p)
            try:
                import concourse  # noqa: F401
                return
            except ImportError:
                sys.path.remove(p)
    raise ImportError("concourse (bass) not importable")


_ensure_concourse()

import concourse.bass as bass  # noqa: E402
import concourse.mybir as mybir  # noqa: E402
import concourse.tile as tile  # noqa: E402
from concourse import bacc  # noqa: E402
from concourse.bass_utils import run_bass_kernel_spmd  # noqa: E402

F32 = mybir.dt.float32
F32R = mybir.dt.float32r
BF16 = mybir.dt.bfloat16
FP8 = mybir.dt.float8e4

C = 128          # channels / hidden dim
B = 4            # batch
N = 4096         # tokens per batch (64*64)
NQ = 2048        # query tokens per core (half batch)
N_CORES = 8
NJT = N // 128   # 32 key chunks of 128
N_IB = NQ // 512  # 4 query blocks of 512
NG = NJT // 2    # 16 key-chunk pairs
SCALE = float(1.0 / np.sqrt(C))

# Schraudolph exp on DVE: i8 = S*EXPA + EXPB, bit pattern read as fp8e4m3
# approximates exp(S*SCALE) (3-bit-mantissa log-linear interpolation).
# EXPB includes +0.5 so numpy-trunc in CoreSim == round-to-nearest on HW
# minus half an ulp; softmax normalization cancels the systematic factor.
EXPA = float(8.0 * SCALE / np.log(2.0))
EXPB = 56.5

# den pair-accumulation engine split per block (16 key-chunk pairs):
# Pool (gpsimd, SBUF-only, otherwise idle) and DVE accumulate e pairs
# elementwise into per-engine Dsum tiles; a few pairs stay on the PE as
# fp8 DoubleRow ones-matmuls. Block 0 keeps the PE out of den so the
# PSUM aux rotation stays free for the setup-phase k/v staging.
def _envset(name, default):
    v = os.environ.get(name)
    if v is None:
        return frozenset(default)
    if not v.strip():
        return frozenset()
    return frozenset(int(x) for x in v.split(","))


POOL_DEN = _envset("K_POOL_DEN", (0, 4, 8, 12))
PE_DEN = _envset("K_PE_DEN", (2, 6, 10, 14))
DEN_LAG = int(os.environ.get("K_DEN_LAG", "2"))


def exp_engine(ib, j):
    """Engine for the exp of key half-chunk j (0..31) of query block ib.

    ACT (exact exp LUT) and DVE (Schraudolph tensor_scalar) split the
    softmax exp work; ACT carries the setup-phase k/v/q conversions during
    block 0, so it gets a smaller share there; DVE carries den-adds and the
    epilogues, so ACT takes most exps in blocks 1-3.
    """
    if ib == 0:
        return "dve" if j % 2 == 1 else "act"
    return "dve" if j % 4 == 2 else "act"


def build_nc(repeats=1):
    nc = bacc.Bacc("TRN2", target_bir_lowering=False, debug=False,
                   num_devices=N_CORES)

    x1bf = nc.declare_dram_parameter("x1bf", [C, NQ], BF16, isOutput=False)
    x2f = nc.declare_dram_parameter("x2f", [C, N], BF16, isOutput=False)
    wqT = nc.declare_dram_parameter("wqT", [C, C], BF16, isOutput=False)
    wkT = nc.declare_dram_parameter("wkT", [C, C], BF16, isOutput=False)
    wvT = nc.declare_dram_parameter("wvT", [C, C], BF16, isOutput=False)
    bqv = nc.declare_dram_parameter("bqv", [C, 2], F32, isOutput=False)
    out = nc.declare_dram_parameter("out", [C, NQ], F32, isOutput=True)

    with tile.TileContext(nc) as tc:
        with (
            tc.tile_pool(name="const", bufs=1) as cpool,
            tc.tile_pool(name="persist", bufs=1) as ppool,
            tc.tile_pool(name="work", bufs=2) as wpool,
            tc.tile_pool(name="ps_s", bufs=4, space="PSUM") as ps_s,
            tc.tile_pool(name="ps_o", bufs=2, space="PSUM") as ps_o,
            tc.tile_pool(name="ps_aux", bufs=2, space="PSUM") as ps_aux,
        ):
            pools = (cpool, ppool, wpool, ps_s, ps_o, ps_aux)
            if repeats == 1:
                _build_body(nc, pools, x1bf, x2f, wqT, wkT, wvT, bqv, out)
            else:
                hints = (mybir.EngineType.PE, mybir.EngineType.Activation,
                         mybir.EngineType.DVE, mybir.EngineType.SP,
                         mybir.EngineType.Pool)
                with tc.For_i(0, repeats, 1, hint_engines=hints):
                    _build_body(nc, pools, x1bf, x2f, wqT, wkT, wvT, bqv,
                                out)

    nc.compile()
    return nc


def _build_body(nc, pools, x1bf, x2f, wqT, wkT, wvT, bqv, out):
    (cpool, ppool, wpool, ps_s, ps_o, ps_aux) = pools
    LAG = int(os.environ.get("K_LAG", "3"))
    DIAG_NO_DEN = os.environ.get("K_NO_DEN") == "1"
    DIAG_NO_PV = os.environ.get("K_NO_PV") == "1"
    DIAG_NO_EXP = os.environ.get("K_NO_EXP") == "1"
    DIAG_NO_S = os.environ.get("K_NO_S") == "1"

    # ---- constants: weights + biases + f32r ones for den broadcast -----
    w_b = {}
    for wname, wdram in (("wq", wqT), ("wk", wkT), ("wv", wvT)):
        wb = cpool.tile([C, C], BF16, tag=f"{wname}b", name=f"{wname}b")
        nc.sync.dma_start(wb[:], wdram[:])
        w_b[wname] = wb
    ones_r = cpool.tile([C, C], BF16, tag="ones_r", name="ones_r")
    nc.vector.memset(ones_r[:], 1.0)
    # DoubleRow ones with M=128 for the PE-resident den pairs: the DR
    # matmul then writes the denominator pre-broadcast across partitions.
    ones_dr = cpool.tile([C, 2, C], FP8, tag="ones_dr", name="ones_dr")
    nc.vector.memset(ones_dr[:].rearrange("p two n -> p (two n)"), 1.0)
    bias_t = cpool.tile([C, 2], F32, tag="bias", name="bias_t")
    bias_sb = {"bq": bias_t[:, 0:1], "bv": bias_t[:, 1:2]}

    # ---- input DMAs: x2 chunk ci feeds k/v; x1 chunk ib feeds q(ib) ----
    x1b = [None] * N_IB
    x2b = [None] * 8
    order = [("x2", 0), ("x1", 0), ("x2", 1), ("x2", 2), ("x1", 1),
             ("x2", 3), ("x2", 4), ("x1", 2), ("x2", 5), ("x2", 6),
             ("x1", 3), ("x2", 7)]
    bias_loaded = [False]
    for kind, ci in order:
        if kind == "x1":
            xb = ppool.tile([C, 512], BF16, tag=f"x1b{ci}", name=f"x1b{ci}")
            nc.sync.dma_start(xb[:], x1bf[:, ci * 512:(ci + 1) * 512])
            x1b[ci] = xb
        else:
            xb = ppool.tile([C, 512], BF16, tag=f"x2b{ci}", name=f"x2b{ci}")
            nc.sync.dma_start(xb[:], x2f[:, ci * 512:(ci + 1) * 512])
            x2b[ci] = xb
        if not bias_loaded[0] and ci >= 1:
            nc.sync.dma_start(bias_t[:], bqv[:])
            bias_loaded[0] = True

    qb = [None] * N_IB
    x1c = [None] * N_IB
    kb = []
    vpair = []

    def emit_qproj(ib):
        q_ps = ps_o.tile([C, 512], F32, tag="o", name=f"qps{ib}")
        nc.tensor.matmul(q_ps[:], w_b["wq"][:], x1b[ib][:],
                         start=True, stop=True)
        qt = ppool.tile([C, 512], BF16, tag=f"qb{ib}", name=f"qb{ib}")
        nc.scalar.activation(qt[:], q_ps[:],
                             mybir.ActivationFunctionType.Identity,
                             bias=bias_sb["bq"], scale=1.0)
        qb[ib] = qt

    def emit_x1c(ib):
        # residual base + v-bias in f32 (v-bias exact after normalization)
        xc = ppool.tile([C, 512], F32, tag=f"x1c{ib}", name=f"x1c{ib}")
        nc.scalar.activation(xc[:], x1b[ib][:],
                             mybir.ActivationFunctionType.Identity,
                             bias=bias_sb["bv"], scale=1.0)
        x1c[ib] = xc

    # pending epilogue of the previous block, injected mid-next-block so
    # the PE queue never waits on the vector-den tail
    pending = [None]

    def attention_ib(ib):
        if qb[ib] is None:
            emit_qproj(ib)
        o_ps = ps_o.tile([C, 512], F32, tag="o", name=f"ops{ib}")
        if DIAG_NO_PV:
            nc.vector.memset(o_ps[:], 1.0)
        epairs = {}
        dsum = {}  # engine -> Dsum tile [128, 2, 512] bf16
        pe_total = len([j for j in range(NG) if j in PE_DEN])
        den_state = {"ps": None, "started": False, "pe_left": pe_total,
                     "has_vec": pe_total < NG}

        pe_den_deferred = []

        def den_pe(jg):
            if den_state["ps"] is None:
                den_state["ps"] = ps_aux.tile(
                    [C, 512], F32, tag="aux", name=f"den{ib}")
            den_state["pe_left"] -= 1
            nc.tensor.matmul(
                den_state["ps"][:], ones_dr[:], epairs[jg][:],
                start=not den_state["started"],
                stop=(not den_state["has_vec"])
                and den_state["pe_left"] == 0,
                perf_mode=mybir.MatmulPerfMode.DoubleRow,
                skip_group_check=True)
            den_state["started"] = True

        def den_pair(jg):
            if DIAG_NO_DEN:
                return
            e_flat = epairs[jg][:].rearrange("p two n -> p (two n)")
            if jg in PE_DEN:
                # block 0's den matmuls are deferred to the block tail so
                # the PSUM aux rotation stays free for the setup staging
                if ib == 0:
                    pe_den_deferred.append(jg)
                else:
                    den_pe(jg)
                return
            eng = "pool" if jg in POOL_DEN else "dve"
            if eng not in dsum:
                t = wpool.tile([128, 2, 512], BF16, tag=f"ds_{eng}", bufs=2,
                               name=f"ds_{eng}{ib}")
                dsum[eng] = t
                t_flat = t[:].rearrange("p two n -> p (two n)")
                if eng == "pool":
                    nc.gpsimd.tensor_copy(t_flat, e_flat)
                else:
                    nc.vector.tensor_copy(t_flat, e_flat)
            else:
                t = dsum[eng]
                t_flat = t[:].rearrange("p two n -> p (two n)")
                if eng == "pool":
                    nc.gpsimd.tensor_add(t_flat, t_flat, e_flat)
                else:
                    nc.vector.tensor_add(t_flat, t_flat, e_flat)

        def emit_s(jg):
            e = wpool.tile([128, 2, 512], FP8, tag="e",
                           bufs=20, name=f"e{ib}_{jg}")
            epairs[jg] = e
            if DIAG_NO_S:
                nc.vector.memset(
                    e[:].rearrange("p two n -> p (two n)"), 1.0)
                return
            for half in range(2):
                j = 2 * jg + half
                s_ps = ps_s.tile([128, 512], F32, tag="s",
                                 name=f"sps{ib}_{j}")
                kt = kb[j // 4][:, (j % 4) * 128:(j % 4) * 128 + 128]
                nc.tensor.matmul(s_ps[:], kt, qb[ib][:],
                                 start=True, stop=True)
                if DIAG_NO_EXP:
                    nc.vector.memset(e[:, half, :].bitcast(mybir.dt.int8), 1)
                    continue
                if exp_engine(ib, j) == "dve":
                    nc.vector.tensor_scalar(
                        e[:, half, :].bitcast(mybir.dt.int8),
                        s_ps[:], EXPA, EXPB,
                        mybir.AluOpType.mult, mybir.AluOpType.add)
                else:
                    nc.scalar.activation(
                        e[:, half, :], s_ps[:],
                        mybir.ActivationFunctionType.Exp,
                        bias=0.0, scale=SCALE)

        def emit_pv(jg):
            if DIAG_NO_PV:
                return
            nc.tensor.matmul(
                o_ps[:], vpair[jg], epairs[jg][:],
                start=jg == 0, stop=jg == NG - 1,
                perf_mode=mybir.MatmulPerfMode.DoubleRow)

        for jg in range(NG):
            emit_s(jg)
            if jg >= DEN_LAG:
                den_pair(jg - DEN_LAG)
            if jg >= LAG:
                emit_pv(jg - LAG)
            if jg == 1 and pending[0] is not None:
                # previous block's epilogue: early enough that its o/den
                # PSUM reads are emitted before this block's rotation
                # reclaims those buffers (keeps the PE queue stall-free)
                pending[0]()
                pending[0] = None
            if jg == 4 and x1c[ib] is None:
                emit_x1c(ib)
            yield
        for jg in range(NG - DEN_LAG, NG):
            den_pair(jg)
        for jg in range(NG - LAG, NG):
            emit_pv(jg)
        for jg in pe_den_deferred:
            den_pe(jg)
        for jg in range(NG):
            del epairs[jg]

        def epilogue():
            den_ps = den_state["ps"]
            if den_ps is None:
                den_ps = ps_aux.tile([C, 512], F32, tag="aux",
                                     name=f"den{ib}")
            if DIAG_NO_DEN:
                nc.vector.memset(den_ps[:], 1.0)
            else:
                # partition-reduce + broadcast each engine Dsum with an
                # f32r ones-matmul (1 cyc/col at 512 cols), accumulating
                # on top of any PE-resident DoubleRow den pairs
                parts = []
                for eng in ("dve", "pool"):
                    if eng in dsum:
                        parts.append(dsum[eng])
                last = (len(parts) - 1, 1)
                for pi, t in enumerate(parts):
                    for h in range(2):
                        nc.tensor.matmul(
                            den_ps[:], ones_r[:], t[:, h, :],
                            start=not den_state["started"],
                            stop=(pi, h) == last,
                            skip_group_check=True)
                        den_state["started"] = True
            rbs = wpool.tile([C, 512], F32, tag="rbs", bufs=2,
                             name=f"rbs{ib}")
            nc.vector.reciprocal(rbs[:], den_ps[:])
            ob = wpool.tile([C, 512], F32, tag="ob", bufs=2, name=f"ob{ib}")
            nc.vector.tensor_mul(ob[:], o_ps[:], rbs[:])
            nc.gpsimd.tensor_add(ob[:], ob[:], x1c[ib][:])
            nc.sync.dma_start(out[:, ib * 512:(ib + 1) * 512], ob[:])

        pending[0] = epilogue
        yield

    # ---- setup: per x2 chunk, k projection + v tiles, advancing block 0
    # attention as chunks land (pair jg needs k/v subchunks 2jg, 2jg+1)
    gen0_holder = [None]
    for ci in range(8):
        k_ps = ps_aux.tile([C, 512], F32, tag="aux", name=f"kps{ci}")
        nc.tensor.matmul(k_ps[:], w_b["wk"][:], x2b[ci][:],
                         start=True, stop=True)
        kt = ppool.tile([C, 512], BF16, tag=f"kb{ci}", name=f"kb{ci}")
        nc.scalar.activation(kt[:], k_ps[:],
                             mybir.ActivationFunctionType.Copy,
                             bias=0.0, scale=1.0)
        kb.append(kt)
        # v chunks of this ci as one PSUM quad [128, 4, C] (1 bank),
        # converted to fp8 with a single wide ACT copy
        v_ps = ps_aux.tile([128, 4, C], F32, tag="aux", name=f"vps{ci}")
        for t in range(4):
            lhs = x2b[ci][:, t * 128:t * 128 + 128]
            nc.tensor.matmul(v_ps[:, t, :], lhs, w_b["wv"][:],
                             start=True, stop=True)
        vq = ppool.tile([128, 4, C], FP8, tag=f"vq{ci}", name=f"vq{ci}")
        nc.scalar.activation(vq[:], v_ps[:],
                             mybir.ActivationFunctionType.Copy,
                             bias=0.0, scale=1.0)
        vpair.append(vq[:, 0:2, :])
        vpair.append(vq[:, 2:4, :])
        if gen0_holder[0] is None:
            gen0_holder[0] = attention_ib(0)
        next(gen0_holder[0], None)
        next(gen0_holder[0], None)

    # ---- attention main loop --------------------------------------------
    gen0 = gen0_holder[0]
    if gen0 is not None:
        for _ in gen0:
            pass
    for ib2 in (1, 2, 3):
        for _ in attention_ib(ib2):
            pass
    if pending[0] is not None:
        pending[0]()
        pending[0] = None


_NC_CACHE = None


def _get_nc():
    global _NC_CACHE
    if _NC_CACHE is None:
        _NC_CACHE = build_nc()
    return _NC_CACHE


def make_in_maps(x1, x2, wq, bq, wk, bk, wv, bv):
    x1 = np.asarray(x1, np.float32)
    x2 = np.asarray(x2, np.float32)
    t1 = np.ascontiguousarray(x1.reshape(B, C, N))
    t2 = np.ascontiguousarray(x2.reshape(B, C, N))
    import ml_dtypes
    bf = ml_dtypes.bfloat16
    shared = {
        "wqT": np.ascontiguousarray(np.asarray(wq, np.float32).T.astype(bf)),
        "wkT": np.ascontiguousarray(np.asarray(wk, np.float32).T.astype(bf)),
        "wvT": np.ascontiguousarray(np.asarray(wv, np.float32).T.astype(bf)),
        "bqv": np.ascontiguousarray(np.stack(
            [np.asarray(bq, np.float32), np.asarray(bv, np.float32)],
            axis=1)),
    }
    in_maps = []
    for core in range(N_CORES):
        b, h = core // 2, core % 2
        in_maps.append({
            "x1bf": np.ascontiguousarray(
                t1[b][:, h * NQ:(h + 1) * NQ]).astype(bf),
            "x2f": t2[b].astype(bf),
            **shared,
        })
    return in_maps


def assemble_out(results):
    out = np.empty((B, C, N), np.float32)
    for core in range(N_CORES):
        b, h = core // 2, core % 2
        out[b][:, h * NQ:(h + 1) * NQ] = results[core]["out"]
    return out.reshape(B, C, 64, 64)


def kernel(x1, x2, wq, bq, wk, bk, wv, bv):
    nc = _get_nc()
    in_maps = make_in_maps(x1, x2, wq, bq, wk, bk, wv, bv)
    res = run_bass_kernel_spmd(nc, in_maps, list(range(N_CORES)))
    return assemble_out(res.results)
